# revision 11
# baseline (speedup 1.0000x reference)
"""DeepseekV2 MLA decode (matrix-absorbed) on 8 Trainium2 NeuronCores.

Sharding:
  - W_DQ row-sharded (contraction) -> partial cQ -> AllReduce (49KB) -> RMSNorm
    computed redundantly on every core (ln_w is folded into W_QR/W_UQ_UK host-side).
  - W_QR / W_UQ_UK head-sharded (16 of 128 heads per core), stored bf16.
  - AllGather of q in bf16 (per-core [8,16,576] -> [8,8,16,576]).
  - Attention sharded over kv_len (1024 of 8192 positions per core, all 128 heads);
    rope applied to k host-side with *relative* positions (q stays un-roped:
    R(a)q . R(b)k = q . R(b-a)k) and fed pre-transposed as keT [B,64,KVL] bf16;
    softmax exp without max subtraction (scores are O(6)), partial exp-sums via
    activation accum_out; partial (attn, lsum) -> two bf16 AllReduces (batch
    halves, the first overlapping the second half's attention) so every core
    holds the full attention state.
  - W_UV_O COLUMN-sharded (each core computes out[:, 640-col slice] with full
    contraction over all 128 heads) -> no output collective; the host
    concatenates the 8 column slices.
  - DMA counts are kept low (~60/core: the SP sequencer spends ~1.1us per
    dma_start); the W_UV_O stream uses 2.6MB tiles alternating between the SP
    and scalar-engine DGE rings, with a 7-deep prefetch pool (18MB) that fills
    during attention + collectives.

All tensors are quantized to bf16 on the host (tolerance is 2e-2; measured
end-to-end error ~6e-3).  Matmuls accumulate in fp32 PSUM.
"""
import sys

if "/opt/trn_rl_repo" not in sys.path:
    sys.path.insert(0, "/opt/trn_rl_repo")

import numpy as np

N_CORES = 8
B = 8           # batch
H = 5120        # hidden
NH = 128        # heads
QLR = 1536      # q lora rank
ROPE = 64
KVLR = 512
KV = 8192
THETA = 10000.0
SCALE = 192.0 ** -0.5

HL = NH // N_CORES      # 16 local heads
KVL = KV // N_CORES     # 1024 local kv positions
HD = H // N_CORES       # 640 local hidden (stage-1 contraction / out column shard)
KT = KVL // 128         # 8 kv tiles of 128 per core
RT = NH * KVLR // 128   # 512 r-tiles in the output contraction
RG4 = 16                # r-tiles per W_UV_O DMA

_CACHE = {}


def build_nc(sim=False):
    import concourse.bacc as bacc
    import concourse.mybir as mybir
    import concourse.tile as tile

    F32 = mybir.dt.float32
    BF16 = mybir.dt.bfloat16
    AF = mybir.ActivationFunctionType

    nc = bacc.Bacc("TRN2", target_bir_lowering=False, debug=False,
                   num_devices=(1 if sim else N_CORES))

    # ---- per-core inputs ----
    hs = nc.dram_tensor("hs", [B, HD], BF16, kind="ExternalInput")
    wdq = nc.dram_tensor("wdq", [HD, QLR], BF16, kind="ExternalInput")
    wqr = nc.dram_tensor("wqr", [QLR, HL * ROPE], BF16, kind="ExternalInput")
    wuk = nc.dram_tensor("wuk", [QLR, HL * KVLR], BF16, kind="ExternalInput")
    ckv = nc.dram_tensor("ckv", [B, KVL, KVLR], BF16, kind="ExternalInput")
    ket = nc.dram_tensor("ket", [B, ROPE, KVL], BF16, kind="ExternalInput")
    identb = nc.dram_tensor("identb", [128, 128], BF16, kind="ExternalInput")
    wuvo = nc.dram_tensor("wuvo", [NH * KVLR, HD], BF16, kind="ExternalInput")
    out = nc.dram_tensor("out", [B, HD], F32, kind="ExternalOutput")

    RG = [list(range(N_CORES))]

    def coll(kind, op, in_t, out_t):
        if not sim:
            nc.gpsimd.collective_compute(kind, op, replica_groups=RG,
                                         ins=[in_t.opt()], outs=[out_t.opt()])
        elif kind == "AllGather":
            nc.sync.dma_start(out_t[0], in_t[:])
        else:
            nc.sync.dma_start(out_t[:], in_t[:])

    with tile.TileContext(nc) as tc:
        with (
            tc.tile_pool(name="const", bufs=1) as cpool,
            tc.tile_pool(name="dram", bufs=1, space="DRAM") as dram,
        ):
            idb = cpool.tile([128, 128], BF16)
            nc.sync.dma_start(idb[:], identb[:])
            eps = cpool.tile([8, 1], F32)
            nc.vector.memset(eps[:], 1e-6)

            # collective bounce buffers
            cq_ar_in = dram.tile([B, QLR], F32)
            cq_ar_out = dram.tile([B, QLR], F32)
            q_ag_in = dram.tile([B, HL, KVLR + ROPE], BF16)
            q_ag_out = dram.tile([N_CORES, B, HL, KVLR + ROPE], BF16)
            at_ar_in_a = dram.tile([B // 2, NH, KVLR + 1], BF16)
            at_ar_out_a = dram.tile([B // 2, NH, KVLR + 1], BF16)
            at_ar_in_b = dram.tile([B // 2, NH, KVLR + 1], BF16)
            at_ar_out_b = dram.tile([B // 2, NH, KVLR + 1], BF16)

            # =========== Stage 1: cQ = rmsnorm(hs @ W_DQ) ===========
            with (
                tc.tile_pool(name="s1", bufs=1) as s1,
                tc.tile_pool(name="s12ps", bufs=1, space="PSUM") as s1ps,
                tc.tile_pool(name="qnps", bufs=2, space="PSUM") as qnps,
                tc.tile_pool(name="tp1", bufs=1, space="PSUM") as tpp,
                tc.tile_pool(name="wqr_sb", bufs=1) as wqp,
                tc.tile_pool(name="wuk_sb", bufs=4) as wkp,
            ):
                hs_sb = s1.tile([B, HD], BF16)
                nc.sync.dma_start(hs_sb[:], hs[:])
                wdq_sb = s1.tile([128, 5, QLR], BF16)
                nc.sync.dma_start(wdq_sb[:], wdq[:].rearrange("(k p) j -> p k j", p=128))
                hsT = s1.tile([128, 5, 8], BF16)
                for k in range(5):
                    tp = tpp.tile([128, 8], BF16, tag="tpb")
                    nc.tensor.transpose(tp[:], hs_sb[:, k * 128:(k + 1) * 128], idb[0:8, 0:8])
                    nc.vector.tensor_copy(hsT[:, k, :], tp[:])
                cq_ps = s1ps.tile([8, QLR], F32)
                for n in range(3):
                    for k in range(5):
                        nc.tensor.matmul(
                            cq_ps[:, n * 512:(n + 1) * 512],
                            hsT[:, k, :],
                            wdq_sb[:, k, n * 512:(n + 1) * 512],
                            start=(k == 0), stop=(k == 4),
                        )
                cqraw = s1.tile([8, QLR], F32)
                nc.scalar.copy(cqraw[:], cq_ps[:])
                nc.sync.dma_start(cq_ar_in[:], cqraw[:])
                coll("AllReduce", mybir.AluOpType.add, cq_ar_in, cq_ar_out)
                cqsum = s1.tile([8, QLR], F32)
                nc.sync.dma_start(cqsum[:], cq_ar_out[:])
                # rmsnorm (ln_w folded into the weights host-side)
                sq = s1.tile([8, QLR], F32)
                ssq = s1.tile([8, 1], F32)
                nc.scalar.activation(sq[:], cqsum[:], AF.Square, accum_out=ssq[:])
                sdev = s1.tile([8, 1], F32)
                nc.scalar.activation(sdev[:], ssq[:], AF.Sqrt, bias=eps[:], scale=1.0 / QLR)
                rinv = s1.tile([8, 1], F32)
                nc.vector.reciprocal(rinv[:], sdev[:])
                cqn = s1.tile([8, QLR], BF16)
                nc.vector.tensor_scalar_mul(cqn[:], cqsum[:], rinv[:])
                cqnT = s1.tile([128, 12, 8], BF16)
                for k in range(12):
                    tp = tpp.tile([128, 8], BF16, tag="tpb")
                    nc.tensor.transpose(tp[:], cqn[:, k * 128:(k + 1) * 128], idb[0:8, 0:8])
                    nc.vector.tensor_copy(cqnT[:, k, :], tp[:])

                # =========== Stage 2: q projections for 16 local heads ===========
                qpe_sb = s1.tile([8, HL * ROPE], BF16)
                wq_t = wqp.tile([128, 12, HL * ROPE], BF16)
                nc.sync.dma_start(wq_t[:], wqr[:].rearrange("(kk p) n -> p kk n", p=128))
                for n in range(2):
                    ps_q = qnps.tile([8, 512], F32)
                    for k in range(12):
                        nc.tensor.matmul(ps_q[:], cqnT[:, k, :],
                                         wq_t[:, k, n * 512:(n + 1) * 512],
                                         start=(k == 0), stop=(k == 11))
                    nc.scalar.copy(qpe_sb[:, n * 512:(n + 1) * 512], ps_q[:])
                qn_sb = s1.tile([8, HL * KVLR], BF16)
                for g in range(8):
                    wt = wkp.tile([128, 12, 1024], BF16, tag="wuk")
                    nc.sync.dma_start(
                        wt[:], wuk[:].rearrange("(kk p) n -> p kk n", p=128)[:, :, g * 1024:(g + 1) * 1024]
                    )
                    for hf in range(2):
                        n = g * 2 + hf
                        ps_q = qnps.tile([8, 512], F32)
                        for k in range(12):
                            nc.tensor.matmul(ps_q[:], cqnT[:, k, :],
                                             wt[:, k, hf * 512:(hf + 1) * 512],
                                             start=(k == 0), stop=(k == 11))
                        nc.scalar.copy(qn_sb[:, n * 512:(n + 1) * 512], ps_q[:])
                # deinterleave q_pe (concat-halves permutation, matching host k rope)
                qpe2 = s1.tile([8, HL, ROPE], BF16)
                qv = qpe_sb[:].rearrange("b (h r) -> b h r", h=HL)
                nc.vector.tensor_copy(qpe2[:, :, 0:32], qv[:, :, 0:ROPE:2])
                nc.vector.tensor_copy(qpe2[:, :, 32:64], qv[:, :, 1:ROPE:2])
                # pack q into the allgather buffer
                nc.sync.dma_start(
                    q_ag_in[:, :, 0:KVLR],
                    qn_sb[:].rearrange("b (h l) -> b h l", h=HL),
                )
                nc.sync.dma_start(q_ag_in[:, :, KVLR:KVLR + ROPE], qpe2[:])
                coll("AllGather", mybir.AluOpType.bypass, q_ag_in, q_ag_out)

            # W_UV_O prefetch pool: opened for stages 3+4 so its (big) slots
            # don't fight stage-1/2 SBUF; the scheduler hoists these loads
            # into the attention/collective windows as slots allow.
            with tc.tile_pool(name="wuvo_sb", bufs=7) as wvp:
                # ====== Stage 3: attention over local kv shard, all 128 heads ======
                with (
                    tc.tile_pool(name="s3", bufs=2) as s3,
                    tc.tile_pool(name="s3o", bufs=2) as s3o,
                    tc.tile_pool(name="scps", bufs=2, space="PSUM") as scps,
                    tc.tile_pool(name="atps", bufs=2, space="PSUM") as atps,
                    tc.tile_pool(name="tp3", bufs=2, space="PSUM") as tpp,
                ):
                    for b in range(B):
                        qa = s3.tile([128, KVLR + ROPE], BF16, tag="qa")
                        nc.sync.dma_start(qa[:], q_ag_out[:, b, :, :])
                        # transpose q
                        qnT = s3.tile([128, 4, 128], BF16, tag="qnT")
                        tp = tpp.tile([128, 512], BF16, tag="tpb512")
                        for lc in range(4):
                            nc.tensor.transpose(tp[:, lc * 128:(lc + 1) * 128],
                                                qa[:, lc * 128:(lc + 1) * 128], idb[:])
                        nc.vector.tensor_copy(qnT[:].rearrange("p a b -> p (a b)"), tp[:])
                        qeT = s3.tile([64, 128], BF16, tag="qeT")
                        tpq = tpp.tile([64, 128], BF16, tag="tpb512")
                        nc.tensor.transpose(tpq[:], qa[:, KVLR:KVLR + ROPE], idb[:])
                        nc.vector.tensor_copy(qeT[:], tpq[:])
                        # load ckv tile [128, t, l]
                        ckv_sb = s3.tile([128, KT, KVLR], BF16, tag="ckv")
                        nc.sync.dma_start(ckv_sb[:], ckv[b].rearrange("(t p) l -> p t l", p=128))
                        # transpose ckv -> [l=4x128, kv=KVL]
                        ckvT = s3.tile([128, 4, KVL], BF16, tag="ckvT")
                        for lc in range(4):
                            for g in range(KT // 4):
                                tp = tpp.tile([128, 512], BF16, tag="tpb512")
                                for j in range(4):
                                    t = g * 4 + j
                                    nc.tensor.transpose(tp[:, j * 128:(j + 1) * 128],
                                                        ckv_sb[:, t, lc * 128:(lc + 1) * 128], idb[:])
                                nc.vector.tensor_copy(ckvT[:, lc, g * 512:(g + 1) * 512], tp[:])
                        # roped k (host-side, relative positions, halves layout), pre-transposed
                        keT = s3.tile([64, KVL], BF16, tag="keT")
                        nc.sync.dma_start(keT[:], ket[b])
                        # scores = qn . ckv^T + qe . ke^T   [128h, KVL]
                        sc_ps = scps.tile([128, KVL], F32)
                        for t2 in range(KVL // 512):
                            sl = slice(t2 * 512, (t2 + 1) * 512)
                            for lc in range(4):
                                nc.tensor.matmul(sc_ps[:, sl], qnT[:, lc, :], ckvT[:, lc, sl],
                                                 start=(lc == 0), stop=False)
                            nc.tensor.matmul(sc_ps[:, sl], qeT[:], keT[:, sl],
                                             start=False, stop=True)
                        # probs (unnormalized) + partial lsum
                        attn_sb = s3o.tile([128, KVLR + 1], BF16, tag="attn")
                        lsum = s3o.tile([128, 1], F32, tag="lsum")
                        probs = s3.tile([128, KVL], BF16, tag="probs")
                        nc.scalar.activation(probs[:], sc_ps[:], AF.Exp, scale=SCALE,
                                             accum_out=lsum[:])
                        nc.vector.tensor_copy(attn_sb[:, KVLR:KVLR + 1], lsum[:])
                        # probs^T
                        probsT = s3.tile([128, KT, 128], BF16, tag="probsT")
                        for g in range(KT // 4):
                            tp = tpp.tile([128, 512], BF16, tag="tpb512")
                            for j in range(4):
                                t = g * 4 + j
                                nc.tensor.transpose(tp[:, j * 128:(j + 1) * 128],
                                                    probs[:, t * 128:(t + 1) * 128], idb[:])
                            nc.vector.tensor_copy(
                                probsT[:, g * 4:(g + 1) * 4, :].rearrange("p a b -> p (a b)"), tp[:])
                        # attn partial = probs^T . ckv  [128h, KVLR]
                        at_ps = atps.tile([128, KVLR], F32)
                        for t in range(KT):
                            nc.tensor.matmul(at_ps[:], probsT[:, t, :], ckv_sb[:, t, :],
                                             start=(t == 0), stop=(t == KT - 1))
                        nc.vector.tensor_copy(attn_sb[:, 0:KVLR], at_ps[:])
                        if b < 4:
                            nc.sync.dma_start(at_ar_in_a[b], attn_sb[:])
                        else:
                            nc.sync.dma_start(at_ar_in_b[b - 4], attn_sb[:])
                        if b == 3:
                            # first-half partials reduce while batches 4-7 compute
                            coll("AllReduce", mybir.AluOpType.add, at_ar_in_a, at_ar_out_a)
                    coll("AllReduce", mybir.AluOpType.add, at_ar_in_b, at_ar_out_b)

                # == Stage 4: out[:, col shard] = (attn/lsum) @ W_UV_O[:, cols] ==
                with (
                    tc.tile_pool(name="s4", bufs=1) as s4,
                    tc.tile_pool(name="s4b", bufs=2) as s4b,
                    tc.tile_pool(name="oaps", bufs=1, space="PSUM") as oaps,
                    tc.tile_pool(name="tp4", bufs=2, space="PSUM") as tpp,
                ):
                    aT = s4.tile([128, RT, 8], BF16)
                    for b in range(B):
                        av = s4b.tile([128, KVLR + 1], BF16, tag="av")
                        src = at_ar_out_a[b] if b < 4 else at_ar_out_b[b - 4]
                        nc.sync.dma_start(av[:], src)
                        linv = s4b.tile([128, 1], F32, tag="linv")
                        nc.vector.reciprocal(linv[:], av[:, KVLR:KVLR + 1])
                        osc = s4b.tile([128, KVLR], BF16, tag="osc")
                        nc.vector.tensor_scalar_mul(osc[:], av[:, 0:KVLR], linv[:])
                        tp = tpp.tile([128, 512], BF16, tag="tpb512")
                        for lc in range(4):
                            nc.tensor.transpose(tp[:, lc * 128:(lc + 1) * 128],
                                                osc[:, lc * 128:(lc + 1) * 128], idb[:])
                        # tp is [128l, (lc h)]; aT wants [128l, (h lc), b]
                        nc.vector.tensor_copy(
                            aT[:, :, b].rearrange("p (h l) -> p l h", l=4),
                            tp[:].rearrange("p (l h) -> p l h", l=4))
                    outp = s4.tile([8, HD], F32)
                    o_psA = oaps.tile([8, 512], F32)
                    o_psB = oaps.tile([8, HD - 512], F32)
                    for g in range(RT // RG4):
                        wt = wvp.tile([128, RG4, HD], BF16, tag="wuvo")
                        eng = nc.sync if g % 2 == 0 else nc.scalar
                        eng.dma_start(
                            wt[:], wuvo[:].rearrange("(r p) n -> p r n", p=128)[:, g * RG4:(g + 1) * RG4, :])
                        for j in range(RG4):
                            r = g * RG4 + j
                            nc.tensor.matmul(o_psA[:], aT[:, r, :], wt[:, j, 0:512],
                                             start=(r == 0), stop=(r == RT - 1))
                            nc.tensor.matmul(o_psB[:], aT[:, r, :], wt[:, j, 512:HD],
                                             start=(r == 0), stop=(r == RT - 1))
                    nc.scalar.copy(outp[:, 0:512], o_psA[:])
                    nc.scalar.copy(outp[:, 512:HD], o_psB[:])
                    nc.sync.dma_start(out[:], outp[:])

    nc.compile()
    return nc


def make_in_maps(hidden_states, compressed_kv_normed_cache, k_pe_cache,
                 W_DQ, ln_w, W_QR, W_UQ_UK, W_UV_O):
    import ml_dtypes
    f32 = np.float32
    bf16 = ml_dtypes.bfloat16
    hidden_states = np.asarray(hidden_states, f32).astype(bf16)
    ckv = np.asarray(compressed_kv_normed_cache, f32).astype(bf16)
    kpe = np.asarray(k_pe_cache, f32)
    W_DQ = np.asarray(W_DQ, f32).astype(bf16)
    ln_w = np.asarray(ln_w, f32)
    W_QR = (np.asarray(W_QR, f32) * ln_w[:, None]).astype(bf16)
    W_UQ_UK = (np.asarray(W_UQ_UK, f32) * ln_w[:, None]).astype(bf16)
    W_UV_O = np.asarray(W_UV_O, f32).astype(bf16)

    # rope k host-side with *relative* positions, deinterleaved into halves,
    # then transposed to [B, ROPE, KV] so the kernel can use it directly.
    inv = 1.0 / (THETA ** (np.arange(0, ROPE, 2, dtype=np.float64) / ROPE))
    rel = (np.arange(KV, dtype=np.float64) - (KV - 1))[:, None] * inv[None, :]
    cost = np.cos(rel).astype(f32)
    sint = np.sin(rel).astype(f32)
    ev, od = kpe[:, :, 0::2], kpe[:, :, 1::2]
    ke = np.concatenate([ev * cost - od * sint, ev * sint + od * cost], axis=-1)
    keT = ke.transpose(0, 2, 1).astype(bf16)          # [B, ROPE, KV]

    identb = np.eye(128, dtype=bf16)

    c = np.ascontiguousarray
    in_maps = []
    for ci in range(N_CORES):
        in_maps.append({
            "hs": c(hidden_states[:, ci * HD:(ci + 1) * HD]),
            "wdq": c(W_DQ[ci * HD:(ci + 1) * HD, :]),
            "wqr": c(W_QR[:, ci * HL * ROPE:(ci + 1) * HL * ROPE]),
            "wuk": c(W_UQ_UK[:, ci * HL * KVLR:(ci + 1) * HL * KVLR]),
            "ckv": c(ckv[:, ci * KVL:(ci + 1) * KVL, :]),
            "ket": c(keT[:, :, ci * KVL:(ci + 1) * KVL]),
            "identb": identb,
            "wuvo": c(W_UV_O[:, ci * HD:(ci + 1) * HD]),
        })
    return in_maps


def kernel(**inputs) -> np.ndarray:
    from concourse import bass_utils

    if "nc" not in _CACHE:
        _CACHE["nc"] = build_nc()
    nc = _CACHE["nc"]
    in_maps = make_in_maps(**inputs)
    res = bass_utils.run_bass_kernel_spmd(nc, in_maps, core_ids=list(range(N_CORES)))
    # out is column-sharded: core ci holds out[:, ci*HD:(ci+1)*HD]
    return np.concatenate(
        [np.asarray(res.results[ci]["out"], np.float32) for ci in range(N_CORES)],
        axis=1)


# revision 21
# speedup vs baseline: 1.2053x; 1.2053x over previous
"""DeepseekV2 MLA decode (matrix-absorbed) on 8 Trainium2 NeuronCores.

Sharding:
  - W_DQ row-sharded (contraction) -> partial cQ -> AllReduce (49KB) -> RMSNorm
    computed redundantly on every core (ln_w is folded into W_QR/W_UQ_UK host-side).
  - W_QR / W_UQ_UK head-sharded (16 of 128 heads per core), stored bf16.
  - AllGather of q in bf16 (per-core [8,16,576] -> [8,8,16,576]).
  - Attention sharded over kv_len (1024 of 8192 positions per core, all 128 heads);
    rope applied to k host-side with *relative* positions (q stays un-roped:
    R(a)q . R(b)k = q . R(b-a)k) and fed pre-transposed as keT [B,64,KVL] bf16;
    softmax exp without max subtraction (scores are O(6)), partial exp-sums via
    activation accum_out; partial (attn, lsum) -> one bf16 AllReduce so every
    core holds the full attention output.
  - W_UV_O COLUMN-sharded (each core computes out[:, 640-col slice] with full
    contraction over all 128 heads) -> no output collective; the host
    concatenates the 8 column slices.
  - DMA counts are kept low (~80/core vs 450 naive: the SP sequencer spends
    ~1.1us per dma_start) and the W_UV_O stream issues from the scalar-engine
    DGE ring so it never queues behind the SP ring.

All big tensors are quantized to bf16 on the host (tolerance is 2e-2; measured
end-to-end error ~4e-3).  Matmuls accumulate in fp32 PSUM.
"""
import sys

if "/opt/trn_rl_repo" not in sys.path:
    sys.path.insert(0, "/opt/trn_rl_repo")

import numpy as np

N_CORES = 8
B = 8           # batch
H = 5120        # hidden
NH = 128        # heads
QLR = 1536      # q lora rank
ROPE = 64
KVLR = 512
KV = 8192
THETA = 10000.0
SCALE = 192.0 ** -0.5

HL = NH // N_CORES      # 16 local heads
KVL = KV // N_CORES     # 1024 local kv positions
HD = H // N_CORES       # 640 local hidden (stage-1 contraction / out column shard)
KT = KVL // 128         # 8 kv tiles of 128 per core
RT = NH * KVLR // 128   # 512 r-tiles in the output contraction

_CACHE = {}


def build_nc(sim=False):
    import concourse.bacc as bacc
    import concourse.mybir as mybir
    import concourse.tile as tile

    F32 = mybir.dt.float32
    F32R = mybir.dt.float32r
    BF16 = mybir.dt.bfloat16
    AF = mybir.ActivationFunctionType

    nc = bacc.Bacc("TRN2", target_bir_lowering=False, debug=False,
                   num_devices=(1 if sim else N_CORES))

    # ---- per-core inputs ----
    hs = nc.dram_tensor("hs", [B, HD], BF16, kind="ExternalInput")
    wdq = nc.dram_tensor("wdq", [HD, QLR], BF16, kind="ExternalInput")
    wqr = nc.dram_tensor("wqr", [QLR, HL * ROPE], BF16, kind="ExternalInput")
    wuk = nc.dram_tensor("wuk", [QLR, HL * KVLR], BF16, kind="ExternalInput")
    ckv = nc.dram_tensor("ckv", [B, KVL, KVLR], BF16, kind="ExternalInput")
    ket = nc.dram_tensor("ket", [B, ROPE, KVL], BF16, kind="ExternalInput")
    identb = nc.dram_tensor("identb", [128, 128], BF16, kind="ExternalInput")
    wuvo = nc.dram_tensor("wuvo", [NH * KVLR, HD], BF16, kind="ExternalInput")
    out = nc.dram_tensor("out", [B, HD], F32, kind="ExternalOutput")

    RG = [list(range(N_CORES))]

    def coll(kind, op, in_t, out_t):
        if not sim:
            nc.gpsimd.collective_compute(kind, op, replica_groups=RG,
                                         ins=[in_t.opt()], outs=[out_t.opt()])
        elif kind == "AllGather":
            nc.sync.dma_start(out_t[0], in_t[:])
        else:
            nc.sync.dma_start(out_t[:], in_t[:])

    with tile.TileContext(nc) as tc:
        with (
            tc.tile_pool(name="const", bufs=1) as cpool,
            tc.tile_pool(name="dram", bufs=1, space="DRAM") as dram,
            tc.tile_pool(name="wuvo_sb", bufs=6) as wvp,
        ):
            idb = cpool.tile([128, 128], BF16)
            nc.sync.dma_start(idb[:], identb[:])
            eps = cpool.tile([8, 1], F32)
            nc.vector.memset(eps[:], 1e-6)

            # collective bounce buffers
            cq_ar_in = dram.tile([B, QLR], F32)
            cq_ar_out = dram.tile([B, QLR], F32)
            q_ag_in = dram.tile([B, HL, KVLR + ROPE], BF16)
            q_ag_out = dram.tile([N_CORES, B, HL, KVLR + ROPE], BF16)
            at_ar_in = dram.tile([B, NH, KVLR + 1], BF16)
            at_ar_out = dram.tile([B, NH, KVLR + 1], BF16)

            # =========== Stage 1: cQ = rmsnorm(hs @ W_DQ) ===========
            with (
                tc.tile_pool(name="s1", bufs=1) as s1,
                tc.tile_pool(name="s12ps", bufs=1, space="PSUM") as s1ps,
                tc.tile_pool(name="qnps", bufs=2, space="PSUM") as qnps,
                tc.tile_pool(name="tp1", bufs=1, space="PSUM") as tpp,
                tc.tile_pool(name="wqr_sb", bufs=1) as wqp,
                tc.tile_pool(name="wuk_sb", bufs=5) as wkp,
            ):
                hs_sb = s1.tile([B, HD], BF16)
                nc.sync.dma_start(hs_sb[:], hs[:])
                wdq_sb = s1.tile([128, 5, QLR], BF16)
                nc.sync.dma_start(wdq_sb[:], wdq[:].rearrange("(k p) j -> p k j", p=128))
                hsT = s1.tile([128, 5, 8], BF16)
                for k in range(5):
                    tp = tpp.tile([128, 8], BF16, tag="tpb")
                    nc.tensor.transpose(tp[:], hs_sb[:, k * 128:(k + 1) * 128], idb[0:8, 0:8])
                    nc.vector.tensor_copy(hsT[:, k, :], tp[:])
                cq_ps = s1ps.tile([8, QLR], F32)
                for n in range(3):
                    for k in range(5):
                        nc.tensor.matmul(
                            cq_ps[:, n * 512:(n + 1) * 512],
                            hsT[:, k, :],
                            wdq_sb[:, k, n * 512:(n + 1) * 512],
                            start=(k == 0), stop=(k == 4),
                        )
                cqraw = s1.tile([8, QLR], F32)
                nc.scalar.copy(cqraw[:], cq_ps[:])
                nc.sync.dma_start(cq_ar_in[:], cqraw[:])
                coll("AllReduce", mybir.AluOpType.add, cq_ar_in, cq_ar_out)
                cqsum = s1.tile([8, QLR], F32)
                nc.sync.dma_start(cqsum[:], cq_ar_out[:])
                # rmsnorm (ln_w folded into the weights host-side)
                sq = s1.tile([8, QLR], F32)
                ssq = s1.tile([8, 1], F32)
                nc.scalar.activation(sq[:], cqsum[:], AF.Square, accum_out=ssq[:])
                sdev = s1.tile([8, 1], F32)
                nc.scalar.activation(sdev[:], ssq[:], AF.Sqrt, bias=eps[:], scale=1.0 / QLR)
                rinv = s1.tile([8, 1], F32)
                nc.vector.reciprocal(rinv[:], sdev[:])
                cqn = s1.tile([8, QLR], BF16)
                nc.vector.tensor_scalar_mul(cqn[:], cqsum[:], rinv[:])
                cqnT = s1.tile([128, 12, 8], BF16)
                for k in range(12):
                    tp = tpp.tile([128, 8], BF16, tag="tpb")
                    nc.tensor.transpose(tp[:], cqn[:, k * 128:(k + 1) * 128], idb[0:8, 0:8])
                    nc.vector.tensor_copy(cqnT[:, k, :], tp[:])

                # =========== Stage 2: q projections for 16 local heads ===========
                qpe_sb = s1.tile([8, HL * ROPE], BF16)
                wq_t = wqp.tile([128, 12, HL * ROPE], BF16)
                nc.sync.dma_start(wq_t[:], wqr[:].rearrange("(kk p) n -> p kk n", p=128))
                for n in range(2):
                    ps_q = qnps.tile([8, 512], F32)
                    for k in range(12):
                        nc.tensor.matmul(ps_q[:], cqnT[:, k, :],
                                         wq_t[:, k, n * 512:(n + 1) * 512],
                                         start=(k == 0), stop=(k == 11))
                    nc.scalar.copy(qpe_sb[:, n * 512:(n + 1) * 512], ps_q[:])
                qn_sb = s1.tile([8, HL * KVLR], BF16)
                for n in range(16):
                    wt = wkp.tile([128, 12, 512], BF16, tag="wuk")
                    nc.sync.dma_start(
                        wt[:], wuk[:].rearrange("(kk p) n -> p kk n", p=128)[:, :, n * 512:(n + 1) * 512]
                    )
                    ps_q = qnps.tile([8, 512], F32)
                    for k in range(12):
                        nc.tensor.matmul(ps_q[:], cqnT[:, k, :], wt[:, k, :],
                                         start=(k == 0), stop=(k == 11))
                    nc.scalar.copy(qn_sb[:, n * 512:(n + 1) * 512], ps_q[:])
                # deinterleave q_pe (concat-halves permutation, matching host k rope)
                qpe2 = s1.tile([8, HL, ROPE], BF16)
                qv = qpe_sb[:].rearrange("b (h r) -> b h r", h=HL)
                nc.vector.tensor_copy(qpe2[:, :, 0:32], qv[:, :, 0:ROPE:2])
                nc.vector.tensor_copy(qpe2[:, :, 32:64], qv[:, :, 1:ROPE:2])
                # pack q into the allgather buffer
                nc.sync.dma_start(
                    q_ag_in[:, :, 0:KVLR],
                    qn_sb[:].rearrange("b (h l) -> b h l", h=HL),
                )
                nc.sync.dma_start(q_ag_in[:, :, KVLR:KVLR + ROPE], qpe2[:])
                coll("AllGather", mybir.AluOpType.bypass, q_ag_in, q_ag_out)

            # =========== Stage 3: attention over local kv shard, all 128 heads ===========
            with (
                tc.tile_pool(name="s3", bufs=2) as s3,
                tc.tile_pool(name="s3o", bufs=2) as s3o,
                tc.tile_pool(name="scps", bufs=2, space="PSUM") as scps,
                tc.tile_pool(name="atps", bufs=2, space="PSUM") as atps,
                tc.tile_pool(name="tp3", bufs=2, space="PSUM") as tpp,
            ):
                for b in range(B):
                    qa = s3.tile([128, KVLR + ROPE], BF16, tag="qa")
                    nc.sync.dma_start(qa[:], q_ag_out[:, b, :, :])
                    # transpose q
                    qnT = s3.tile([128, 4, 128], BF16, tag="qnT")
                    tp = tpp.tile([128, 512], BF16, tag="tpb512")
                    for lc in range(4):
                        nc.tensor.transpose(tp[:, lc * 128:(lc + 1) * 128],
                                            qa[:, lc * 128:(lc + 1) * 128], idb[:])
                    nc.vector.tensor_copy(qnT[:].rearrange("p a b -> p (a b)"), tp[:])
                    qeT = s3.tile([64, 128], BF16, tag="qeT")
                    tpq = tpp.tile([64, 128], BF16, tag="tpb512")
                    nc.tensor.transpose(tpq[:], qa[:, KVLR:KVLR + ROPE], idb[:])
                    nc.vector.tensor_copy(qeT[:], tpq[:])
                    # load ckv tile [128, t, l]
                    ckv_sb = s3.tile([128, KT, KVLR], BF16, tag="ckv")
                    nc.sync.dma_start(ckv_sb[:], ckv[b].rearrange("(t p) l -> p t l", p=128))
                    # transpose ckv -> [l=4x128, kv=KVL]
                    ckvT = s3.tile([128, 4, KVL], BF16, tag="ckvT")
                    for lc in range(4):
                        for g in range(KT // 4):
                            tp = tpp.tile([128, 512], BF16, tag="tpb512")
                            for j in range(4):
                                t = g * 4 + j
                                nc.tensor.transpose(tp[:, j * 128:(j + 1) * 128],
                                                    ckv_sb[:, t, lc * 128:(lc + 1) * 128], idb[:])
                            nc.vector.tensor_copy(ckvT[:, lc, g * 512:(g + 1) * 512], tp[:])
                    # roped k (host-side, relative positions, halves layout), pre-transposed
                    keT = s3.tile([64, KVL], BF16, tag="keT")
                    nc.sync.dma_start(keT[:], ket[b])
                    # scores = qn . ckv^T + qe . ke^T   [128h, KVL]
                    sc_ps = scps.tile([128, KVL], F32)
                    for t2 in range(KVL // 512):
                        sl = slice(t2 * 512, (t2 + 1) * 512)
                        for lc in range(4):
                            nc.tensor.matmul(sc_ps[:, sl], qnT[:, lc, :], ckvT[:, lc, sl],
                                             start=(lc == 0), stop=False)
                        nc.tensor.matmul(sc_ps[:, sl], qeT[:], keT[:, sl],
                                         start=False, stop=True)
                    # probs (unnormalized) + partial lsum
                    attn_sb = s3o.tile([128, KVLR + 1], BF16, tag="attn")
                    lsum = s3o.tile([128, 1], F32, tag="lsum")
                    probs = s3.tile([128, KVL], BF16, tag="probs")
                    nc.scalar.activation(probs[:], sc_ps[:], AF.Exp, scale=SCALE,
                                         accum_out=lsum[:])
                    nc.vector.tensor_copy(attn_sb[:, KVLR:KVLR + 1], lsum[:])
                    # probs^T
                    probsT = s3.tile([128, KT, 128], BF16, tag="probsT")
                    for g in range(KT // 4):
                        tp = tpp.tile([128, 512], BF16, tag="tpb512")
                        for j in range(4):
                            t = g * 4 + j
                            nc.tensor.transpose(tp[:, j * 128:(j + 1) * 128],
                                                probs[:, t * 128:(t + 1) * 128], idb[:])
                        nc.vector.tensor_copy(
                            probsT[:, g * 4:(g + 1) * 4, :].rearrange("p a b -> p (a b)"), tp[:])
                    # attn partial = probs^T . ckv  [128h, KVLR]
                    at_ps = atps.tile([128, KVLR], F32)
                    for t in range(KT):
                        nc.tensor.matmul(at_ps[:], probsT[:, t, :], ckv_sb[:, t, :],
                                         start=(t == 0), stop=(t == KT - 1))
                    nc.vector.tensor_copy(attn_sb[:, 0:KVLR], at_ps[:])
                    nc.sync.dma_start(at_ar_in[b], attn_sb[:])
                # sum partial (attn, lsum) across the 8 kv shards; every core
                # ends up with the full [B, NH, 513] attention state.
                coll("AllReduce", mybir.AluOpType.add, at_ar_in, at_ar_out)

            # ===== Stage 4: out[:, col shard] = (attn/lsum) @ W_UV_O[:, cols] =====
            with (
                tc.tile_pool(name="s4", bufs=1) as s4,
                tc.tile_pool(name="s4b", bufs=2) as s4b,
                tc.tile_pool(name="oaps", bufs=1, space="PSUM") as oaps,
                tc.tile_pool(name="tp4", bufs=2, space="PSUM") as tpp,
            ):
                aT = s4.tile([128, RT, 8], BF16)
                for b in range(B):
                    av = s4b.tile([128, KVLR + 1], BF16, tag="av")
                    nc.sync.dma_start(av[:], at_ar_out[b])
                    linv = s4b.tile([128, 1], F32, tag="linv")
                    nc.vector.reciprocal(linv[:], av[:, KVLR:KVLR + 1])
                    osc = s4b.tile([128, KVLR], BF16, tag="osc")
                    nc.vector.tensor_scalar_mul(osc[:], av[:, 0:KVLR], linv[:])
                    tp = tpp.tile([128, 512], BF16, tag="tpb512")
                    for lc in range(4):
                        nc.tensor.transpose(tp[:, lc * 128:(lc + 1) * 128],
                                            osc[:, lc * 128:(lc + 1) * 128], idb[:])
                    # tp is [128l, (lc h)]; aT wants [128l, (h lc), b]
                    nc.vector.tensor_copy(
                        aT[:, :, b].rearrange("p (h l) -> p l h", l=4),
                        tp[:].rearrange("p (l h) -> p l h", l=4))
                outp = s4.tile([8, HD], F32)
                o_psA = oaps.tile([8, 512], F32)
                o_psB = oaps.tile([8, HD - 512], F32)
                for g in range(RT // 8):
                    wt = wvp.tile([128, 8, HD], BF16, tag="wuvo")
                    nc.scalar.dma_start(
                        wt[:], wuvo[:].rearrange("(r p) n -> p r n", p=128)[:, g * 8:(g + 1) * 8, :])
                    for j in range(8):
                        r = g * 8 + j
                        nc.tensor.matmul(o_psA[:], aT[:, r, :], wt[:, j, 0:512],
                                         start=(r == 0), stop=(r == RT - 1))
                        nc.tensor.matmul(o_psB[:], aT[:, r, :], wt[:, j, 512:HD],
                                         start=(r == 0), stop=(r == RT - 1))
                nc.scalar.copy(outp[:, 0:512], o_psA[:])
                nc.scalar.copy(outp[:, 512:HD], o_psB[:])
                nc.sync.dma_start(out[:], outp[:])

    nc.compile()
    return nc


def make_in_maps(hidden_states, compressed_kv_normed_cache, k_pe_cache,
                 W_DQ, ln_w, W_QR, W_UQ_UK, W_UV_O):
    import ml_dtypes
    f32 = np.float32
    bf16 = ml_dtypes.bfloat16
    hidden_states = np.asarray(hidden_states, f32).astype(bf16)
    ckv = np.asarray(compressed_kv_normed_cache, f32).astype(bf16)
    kpe = np.asarray(k_pe_cache, f32)
    W_DQ = np.asarray(W_DQ, f32).astype(bf16)
    ln_w = np.asarray(ln_w, f32)
    W_QR = (np.asarray(W_QR, f32) * ln_w[:, None]).astype(bf16)
    W_UQ_UK = (np.asarray(W_UQ_UK, f32) * ln_w[:, None]).astype(bf16)
    W_UV_O = np.asarray(W_UV_O, f32).astype(bf16)

    # rope k host-side with *relative* positions, deinterleaved into halves,
    # then transposed to [B, ROPE, KV] so the kernel can use it directly.
    inv = 1.0 / (THETA ** (np.arange(0, ROPE, 2, dtype=np.float64) / ROPE))
    rel = (np.arange(KV, dtype=np.float64) - (KV - 1))[:, None] * inv[None, :]
    cost = np.cos(rel).astype(f32)
    sint = np.sin(rel).astype(f32)
    ev, od = kpe[:, :, 0::2], kpe[:, :, 1::2]
    ke = np.concatenate([ev * cost - od * sint, ev * sint + od * cost], axis=-1)
    keT = ke.transpose(0, 2, 1).astype(bf16)          # [B, ROPE, KV]

    identb = np.eye(128, dtype=bf16)

    c = np.ascontiguousarray
    in_maps = []
    for ci in range(N_CORES):
        in_maps.append({
            "hs": c(hidden_states[:, ci * HD:(ci + 1) * HD]),
            "wdq": c(W_DQ[ci * HD:(ci + 1) * HD, :]),
            "wqr": c(W_QR[:, ci * HL * ROPE:(ci + 1) * HL * ROPE]),
            "wuk": c(W_UQ_UK[:, ci * HL * KVLR:(ci + 1) * HL * KVLR]),
            "ckv": c(ckv[:, ci * KVL:(ci + 1) * KVL, :]),
            "ket": c(keT[:, :, ci * KVL:(ci + 1) * KVL]),
            "identb": identb,
            "wuvo": c(W_UV_O[:, ci * HD:(ci + 1) * HD]),
        })
    return in_maps


def kernel(**inputs) -> np.ndarray:
    from concourse import bass_utils

    if "nc" not in _CACHE:
        _CACHE["nc"] = build_nc()
    nc = _CACHE["nc"]
    in_maps = make_in_maps(**inputs)
    res = bass_utils.run_bass_kernel_spmd(nc, in_maps, core_ids=list(range(N_CORES)))
    # out is column-sharded: core ci holds out[:, ci*HD:(ci+1)*HD]
    return np.concatenate(
        [np.asarray(res.results[ci]["out"], np.float32) for ci in range(N_CORES)],
        axis=1)
